# revision 10
# baseline (speedup 1.0000x reference)
"""Trainium2 Bass kernel for a BERT layer with top-1 MoE feed-forward.

Contract: kernel(**inputs) takes the FULL unsharded inputs (as produced by
setup_inputs()) and returns (layer_output [B,S,H] f32, router_logits [B,E] f32),
matching the reference's return structure.

Strategy (8 NeuronCores): data-parallel over batch — each core owns 2 samples
end-to-end. Attention weights are replicated; expert weights are fetched
per-sample with a data-dependent (register-offset) DMA after on-device routing.
All activations are processed transposed (features on partitions) so every
GEMM consumes pre-transposed operands directly; host pre/post-transposes.
All big GEMMs run as float32r (full PE rate at free-dim >= 256).

Queue discipline: float32r-typed DMAs are issued ONLY on the gpsimd queue
(an f32r DMA on the sync queue alongside dynamic DMAs corrupts unrelated
transfers in-flight); exact fp32/int DMAs and all stores use sync.
"""
import sys

for _p in ('/opt/pypackages', '/opt/trn_rl_repo'):
    if _p not in sys.path:
        sys.path.insert(0, _p)

import numpy as np
import concourse.bass as bass
import concourse.tile as tile
from concourse import bacc, mybir
from concourse.bass_utils import run_bass_kernel_spmd

F32 = mybir.dt.float32
F32R = mybir.dt.float32r
I32 = mybir.dt.int32
AF = mybir.ActivationFunctionType
ALU = mybir.AluOpType
AX = mybir.AxisListType

B, S, H, I, NH, E = 16, 512, 1024, 4096, 16, 8
DH = H // NH            # 64
N_CORES = 8
BPC = B // N_CORES      # samples per core = 2
KO = H // 128           # 8  k-subtiles over H
IO = I // 128           # 32 k-subtiles over I
EPS = 1e-12


def _build():
    nc = bacc.Bacc("TRN2", target_bir_lowering=False, debug=False,
                   num_devices=N_CORES)

    # ---- DRAM I/O ----
    xT = nc.dram_tensor("xT", [KO, 128, BPC * S], F32, kind="ExternalInput").ap()
    wq = nc.dram_tensor("wq", [KO, 128, H], F32, kind="ExternalInput").ap()
    wk = nc.dram_tensor("wk", [KO, 128, H], F32, kind="ExternalInput").ap()
    wv = nc.dram_tensor("wv", [KO, 128, H], F32, kind="ExternalInput").ap()
    wo = nc.dram_tensor("wo", [KO, 128, H], F32, kind="ExternalInput").ap()
    bq = nc.dram_tensor("bq", [KO, 128], F32, kind="ExternalInput").ap()
    bk = nc.dram_tensor("bk", [KO, 128], F32, kind="ExternalInput").ap()
    bvr = nc.dram_tensor("bvr", [1, H], F32, kind="ExternalInput").ap()
    bo = nc.dram_tensor("bo", [KO, 128], F32, kind="ExternalInput").ap()
    g1 = nc.dram_tensor("g1", [KO, 128], F32, kind="ExternalInput").ap()
    be1 = nc.dram_tensor("be1", [KO, 128], F32, kind="ExternalInput").ap()
    g2 = nc.dram_tensor("g2", [KO, 128], F32, kind="ExternalInput").ap()
    be2 = nc.dram_tensor("be2", [KO, 128], F32, kind="ExternalInput").ap()
    gate = nc.dram_tensor("gate", [KO, 128, E], F32, kind="ExternalInput").ap()
    wup = nc.dram_tensor("wup", [E, KO, 128, I], F32, kind="ExternalInput").ap()
    wdn = nc.dram_tensor("wdn", [E, IO, 128, H], F32, kind="ExternalInput").ap()
    bup = nc.dram_tensor("bup", [E, IO, 128], F32, kind="ExternalInput").ap()
    bdn = nc.dram_tensor("bdn", [E, KO, 128], F32, kind="ExternalInput").ap()
    # cst columns: [0:8]=1.0, [8:16]=1/H, [24:152]=1.0 (row-broadcast block)
    cst = nc.dram_tensor("cst", [128, 160], F32, kind="ExternalInput").ap()
    rev = nc.dram_tensor("rev", [1, E], F32, kind="ExternalInput").ap()

    outT = nc.dram_tensor("outT", [KO, 128, BPC * S], F32, kind="ExternalOutput").ap()
    lg_out = nc.dram_tensor("lg_out", [BPC, E], F32, kind="ExternalOutput").ap()

    with tile.TileContext(nc) as tc, \
         tc.tile_pool(name="main", bufs=1) as pool, \
         tc.tile_pool(name="ps", bufs=1, space="PSUM") as pp:

        def psum():
            return pp.tile([128, 512], F32, tag="ps", bufs=8, name="ps")

        # ---- constants / biases ----
        cst_sb = pool.tile([128, 160], F32R, tag="cst", name="cst_sb")
        nc.gpsimd.dma_start(cst_sb[:], cst.bitcast(F32R))
        gate_sb = pool.tile([128, KO, E], F32R, tag="gate", name="gate_sb")
        nc.gpsimd.dma_start(gate_sb[:], gate.rearrange("o p e -> p o e").bitcast(F32R))
        bv_sb = pool.tile([1, H], F32R, tag="bv", name="bv_sb")
        nc.gpsimd.dma_start(bv_sb[:], bvr.bitcast(F32R))

        def load_bias(ap_, tag):
            t = pool.tile([128, KO], F32, tag="bias_" + tag, name="b_" + tag)
            nc.sync.dma_start(t[:], ap_.rearrange("o p -> p o"))
            return t

        bq_sb = load_bias(bq, "q")
        bk_sb = load_bias(bk, "k")
        bo_sb = load_bias(bo, "o")
        g1_sb = load_bias(g1, "g1")
        b1_sb = load_bias(be1, "b1")
        g2_sb = load_bias(g2, "g2")
        b2_sb = load_bias(be2, "b2")
        rev_sb = pool.tile([1, E], F32, tag="rev", name="rev_sb")
        nc.sync.dma_start(rev_sb[:], rev)
        ones_row = cst_sb[0:1, 24:152]        # [1,128] of 1.0 (f32r)
        # activation-bias zero AP (per-partition [128,1] zeros) for Exp etc.
        zb = pool.tile([128, 1], F32, tag="zb", name="zb")
        nc.vector.memset(zb[:], 0.0)
        b8 = pool.tile([1, 1], F32, tag="b8", name="b8")
        nc.vector.memset(b8[:], float(E))

        # ---- xT (activations, transposed) ----
        xT_sb = pool.tile([128, KO, BPC * S], F32R, tag="tagA", name="xT_sb")
        nc.gpsimd.dma_start(xT_sb[:], xT.rearrange("o p n -> p o n").bitcast(F32R))

        ctxT_sb = pool.tile([128, KO, BPC * S], F32R, tag="tagE", name="ctxT_sb")

        def wtile(name):
            return pool.tile([128, KO, 256], F32R, tag="wstrm", bufs=3, name=name)

        # ================= attention: per 256-feature chunk =================
        for wc in range(4):
            msl = slice(wc * 256, (wc + 1) * 256)
            wq_t = wtile(f"wq_{wc}")
            nc.gpsimd.dma_start(wq_t[:], wq[:, :, msl].rearrange("o p m -> p o m").bitcast(F32R))
            wk_t = wtile(f"wk_{wc}")
            nc.gpsimd.dma_start(wk_t[:], wk[:, :, msl].rearrange("o p m -> p o m").bitcast(F32R))
            wv_t = wtile(f"wv_{wc}")
            nc.gpsimd.dma_start(wv_t[:], wv[:, :, msl].rearrange("o p m -> p o m").bitcast(F32R))

            qt_c = pool.tile([128, 2, BPC * S], F32R, tag="qtc", name=f"qt_{wc}")
            kt_c = pool.tile([128, 2, BPC * S], F32R, tag="ktc", name=f"kt_{wc}")
            # V chunk: [p, b, sc, head-local, 64+1]
            v_c = pool.tile([128, BPC, S // 128, 4, DH + 1], F32R, tag="vc",
                            name=f"v_{wc}")

            for (dst, wt, bias) in ((qt_c, wq_t, bq_sb), (kt_c, wk_t, bk_sb)):
                for j2 in range(2):
                    jc = wc * 2 + j2
                    for n in range(2):
                        ps = psum()
                        for k in range(KO):
                            nc.tensor.matmul(ps[:], wt[:, k, j2 * 128:(j2 + 1) * 128],
                                             xT_sb[:, k, n * 512:(n + 1) * 512],
                                             start=(k == 0), stop=(k == KO - 1))
                        nc.scalar.activation(dst[:, j2, n * 512:(n + 1) * 512], ps[:],
                                             AF.Identity, bias=bias[:, jc:jc + 1])

            # V chunk: psum[m = s-chunk (per-sample), n = 256 features]
            for b in range(BPC):
                for sc in range(S // 128):
                    ps = psum()
                    for k in range(KO):
                        nc.tensor.matmul(ps[:, :256],
                                         xT_sb[:, k, b * 512 + sc * 128: b * 512 + (sc + 1) * 128],
                                         wv_t[:, k, :],
                                         start=(k == 0), stop=False)
                    nc.tensor.matmul(ps[:, :256], ones_row, bv_sb[:, msl],
                                     start=False, stop=True)
                    nc.vector.tensor_copy(
                        v_c[:, b, sc, :, 0:DH],
                        ps[:, :256].rearrange("p (h d) -> p h d", h=4))
                # ones column for the fused softmax denominator row
                nc.vector.tensor_copy(
                    v_c[:, b, :, :, DH:DH + 1],
                    cst_sb[:, 0:1, None, None].to_broadcast((128, S // 128, 4, 1)))

            # ---- softmax + ctx for the 4 heads of this chunk ----
            for b in range(BPC):
                bsl = slice(b * 512, (b + 1) * 512)
                for hl in range(4):
                    o2, po = hl // 2, (hl % 2) * 64
                    h_glob = wc * 4 + hl
                    o_h, po_g = h_glob // 2, (h_glob % 2) * 64
                    scps = []
                    for sc in range(S // 128):
                        ps = psum()
                        nc.tensor.matmul(
                            ps[:],
                            kt_c[po:po + 64, o2, b * 512 + sc * 128: b * 512 + (sc + 1) * 128],
                            qt_c[po:po + 64, o2, bsl],
                            start=True, stop=True)
                        scps.append(ps)
                    ex = pool.tile([128, S // 128, 512], F32R, tag="expS", bufs=2,
                                   name=f"ex_{wc}_{b}_{hl}")
                    for sc in range(S // 128):
                        nc.scalar.activation(ex[:, sc, :], scps[sc][:], AF.Exp,
                                             bias=zb[:], scale=1.0 / np.sqrt(DH))
                    cd = psum()
                    for sc in range(S // 128):
                        nc.tensor.matmul(cd[0:DH + 1, :], v_c[:, b, sc, hl, :],
                                         ex[:, sc, :],
                                         start=(sc == 0), stop=(sc == S // 128 - 1))
                    rc = pool.tile([1, 512], F32R, tag="rc", bufs=1,
                                   name=f"rc_{wc}_{b}_{hl}")
                    with nc.allow_low_precision(reason="softmax recip to f32r"):
                        nc.vector.reciprocal(rc[:], cd[DH:DH + 1, :])
                    bc = psum()
                    nc.tensor.matmul(bc[0:DH, :], ones_row[:, 0:DH], rc[:],
                                     start=True, stop=True)
                    bc_sb = pool.tile([DH, 512], F32R, tag="stats", bufs=3,
                                      name=f"bcs_{wc}_{b}_{hl}")
                    nc.scalar.copy(bc_sb[:], bc[0:DH, :])
                    with nc.allow_low_precision(reason="f32r activations"):
                        nc.vector.tensor_mul(ctxT_sb[po_g:po_g + 64, o_h, bsl],
                                             cd[0:DH, :], bc_sb[:])

        # ================= dense + residual (pre-LN1 input) =================
        attnpre_sb = pool.tile([128, KO, BPC * S], F32R, tag="tagB", name="attnpre_sb")
        for wc in range(4):
            msl = slice(wc * 256, (wc + 1) * 256)
            wo_t = wtile(f"wo_{wc}")
            nc.gpsimd.dma_start(wo_t[:], wo[:, :, msl].rearrange("o p m -> p o m").bitcast(F32R))
            for j2 in range(2):
                jc = wc * 2 + j2
                for n in range(2):
                    nsl = slice(n * 512, (n + 1) * 512)
                    ps = psum()
                    for k in range(KO):
                        nc.tensor.matmul(ps[:], wo_t[:, k, j2 * 128:(j2 + 1) * 128],
                                         ctxT_sb[:, k, nsl],
                                         start=(k == 0), stop=(k == KO - 1))
                    res = pool.tile([128, 512], F32, tag="misc", bufs=3,
                                    name=f"res_{jc}_{n}")
                    nc.sync.dma_start(res[:], xT[jc, :, nsl])
                    t = pool.tile([128, 512], F32, tag="misc", bufs=3,
                                  name=f"dt_{jc}_{n}")
                    nc.vector.tensor_add(t[:], ps[:], res[:])
                    nc.scalar.activation(attnpre_sb[:, jc, nsl], t[:],
                                         AF.Identity, bias=bo_sb[:, jc:jc + 1])

        # ================= layernorm (feature dim = partitions) =============
        def layer_norm(in_sb, out_sb, g_sb, b_sb):
            for b in range(BPC):
                bsl = slice(b * 512, (b + 1) * 512)
                mps = psum()
                sps = psum()
                for k in range(KO):
                    nc.tensor.matmul(mps[0:1, :], cst_sb[:, 8 + k, None],
                                     in_sb[:, k, bsl], start=(k == 0),
                                     stop=(k == KO - 1))
                for k in range(KO):
                    sq = pool.tile([128, 512], F32R, tag="misc", bufs=3,
                                   name="sq")
                    nc.scalar.activation(sq[:], in_sb[:, k, bsl], AF.Square)
                    nc.tensor.matmul(sps[0:1, :], cst_sb[:, 8 + k, None], sq[:],
                                     start=(k == 0), stop=(k == KO - 1))
                t2 = pool.tile([1, 512], F32, tag="stats", bufs=3, name="t2")
                nc.scalar.activation(t2[:], mps[0:1, :], AF.Square)
                var = pool.tile([1, 512], F32, tag="stats", bufs=3, name="var")
                nc.vector.tensor_sub(var[:], sps[0:1, :], t2[:])
                nc.vector.tensor_scalar(var[:], var[:], EPS, None, op0=ALU.add)
                rinv = pool.tile([1, 512], F32, tag="stats", bufs=3, name="rinv")
                nc.vector.reciprocal(rinv[:], var[:])
                rstd = pool.tile([1, 512], F32R, tag="stats", bufs=3, name="rstd")
                nc.scalar.activation(rstd[:], rinv[:], AF.Sqrt)
                m2 = pool.tile([1, 512], F32R, tag="stats", bufs=3, name="m2")
                with nc.allow_low_precision(reason="stats"):
                    nc.vector.tensor_mul(m2[:], mps[0:1, :], rstd[:])
                bc_r = psum()
                nc.tensor.matmul(bc_r[:], ones_row, rstd[:], start=True, stop=True)
                bc_m = psum()
                nc.tensor.matmul(bc_m[:], ones_row, m2[:], start=True, stop=True)
                for k in range(KO):
                    t = pool.tile([128, 512], F32, tag="misc", bufs=3, name="lt")
                    nc.vector.tensor_mul(t[:], in_sb[:, k, bsl], bc_r[:])
                    nc.vector.tensor_sub(t[:], t[:], bc_m[:])
                    with nc.allow_low_precision(reason="f32r activations"):
                        nc.vector.tensor_scalar(out_sb[:, k, bsl], t[:],
                                                g_sb[:, k, None], b_sb[:, k, None],
                                                op0=ALU.mult, op1=ALU.add)

        attnout_sb = pool.tile([128, KO, BPC * S], F32R, tag="tagA", name="attnout_sb")
        layer_norm(attnpre_sb, attnout_sb, g1_sb, b1_sb)
        xln_sb = pool.tile([128, KO, BPC * S], F32R, tag="tagB", name="xln_sb")
        layer_norm(attnout_sb, xln_sb, g2_sb, b2_sb)

        # ================= router (per sample) =============================
        choices = []
        for b in range(BPC):
            bsl = slice(b * 512, (b + 1) * 512)
            xbar0 = pool.tile([128, KO], F32, tag="xbar0", bufs=2, name="xbar0")
            nc.vector.reduce_sum(xbar0[:], xln_sb[:, :, bsl], axis=AX.X)
            xbar = pool.tile([128, KO], F32R, tag="xbar", bufs=2, name="xbar")
            with nc.allow_low_precision(reason="router mean"):
                nc.vector.tensor_scalar(xbar[:], xbar0[:], 1.0 / S, None, op0=ALU.mult)
            lp = psum()
            for k in range(KO):
                nc.tensor.matmul(lp[0:1, 0:E], xbar[:, k, None], gate_sb[:, k, :],
                                 start=(k == 0), stop=(k == KO - 1))
            lg_sb = pool.tile([1, E], F32, tag="small", bufs=8, name=f"lg{b}")
            nc.vector.tensor_copy(lg_sb[:], lp[0:1, 0:E])
            nc.sync.dma_start(lg_out[b:b + 1, :], lg_sb[:])
            mx = pool.tile([1, 1], F32, tag="small", bufs=8, name=f"mx{b}")
            nc.vector.reduce_max(mx[:], lg_sb[:], axis=AX.X)
            eq = pool.tile([1, E], F32, tag="small", bufs=8, name=f"eq{b}")
            nc.vector.tensor_scalar(eq[:], lg_sb[:], mx[:], None, op0=ALU.is_equal)
            msk = pool.tile([1, E], F32, tag="small", bufs=8, name=f"msk{b}")
            nc.vector.tensor_mul(msk[:], eq[:], rev_sb[:])
            mm = pool.tile([1, 1], F32, tag="small", bufs=8, name=f"mm{b}")
            nc.vector.reduce_max(mm[:], msk[:], axis=AX.X)
            chf = pool.tile([1, 1], F32, tag="small", bufs=8, name=f"chf{b}")
            nc.scalar.activation(chf[:], mm[:], AF.Identity, bias=b8[:], scale=-1.0)
            chi = pool.tile([1, 1], I32, tag="small", bufs=8, name=f"chi{b}")
            nc.vector.tensor_copy(chi[:], chf[:])
            choices.append(nc.values_load(chi[:], min_val=0, max_val=E - 1,
                                          skip_runtime_bounds_check=True))

        # ================= MoE FFN (per sample, I split in halves) ==========
        for b in range(BPC):
            cb = choices[b]
            bsl = slice(b * 512, (b + 1) * 512)
            bup_sb = pool.tile([128, IO], F32, tag="bup", bufs=2, name=f"bup{b}")
            nc.gpsimd.dma_start(bup_sb[:], bup[bass.ds(cb, 1), :, :].rearrange("e o p -> p (e o)"))
            bdn_sb = pool.tile([128, KO], F32, tag="bdn", bufs=2, name=f"bdn{b}")
            nc.gpsimd.dma_start(bdn_sb[:], bdn[bass.ds(cb, 1), :, :].rearrange("e o p -> p (e o)"))
            acc = pool.tile([128, KO, 512], F32, tag="acc", name=f"acc{b}")
            for half in range(2):
                hT = pool.tile([128, IO // 2, 512], F32R, tag="tagE", name=f"hT{b}_{half}")
                # up projection for this half of I
                for mc in range(8):
                    m0 = half * 2048 + mc * 256
                    wt = wtile(f"wu_{b}_{half}_{mc}")
                    nc.gpsimd.dma_start(
                        wt[:],
                        wup[bass.ds(cb, 1), :, :, m0:m0 + 256]
                        .rearrange("e o p m -> p (e o) m").bitcast(F32R))
                    for ms in range(2):
                        ic = half * 16 + mc * 2 + ms     # global i-chunk 0..31
                        il = mc * 2 + ms                 # local 0..15
                        ps = psum()
                        for k in range(KO):
                            nc.tensor.matmul(ps[:], wt[:, k, ms * 128:(ms + 1) * 128],
                                             xln_sb[:, k, bsl],
                                             start=(k == 0), stop=(k == KO - 1))
                        nc.scalar.activation(hT[:, il, :], ps[:], AF.Gelu,
                                             bias=bup_sb[:, ic:ic + 1])
                # down projection consuming this half (k-chunks 2*half..2*half+1)
                for jm in range(4):
                    m0 = jm * 256
                    wts = []
                    for kc2 in range(2):
                        kc = half * 2 + kc2
                        wt = wtile(f"wd_{b}_{half}_{jm}_{kc2}")
                        nc.gpsimd.dma_start(
                            wt[:],
                            wdn[bass.ds(cb, 1), kc * KO:(kc + 1) * KO, :, m0:m0 + 256]
                            .rearrange("e o p m -> p (e o) m").bitcast(F32R))
                        wts.append(wt)
                    for ms in range(2):
                        jc = jm * 2 + ms
                        ps = psum()
                        for kc2 in range(2):
                            for k in range(KO):
                                nc.tensor.matmul(
                                    ps[:], wts[kc2][:, k, ms * 128:(ms + 1) * 128],
                                    hT[:, kc2 * KO + k, :],
                                    start=(kc2 == 0 and k == 0),
                                    stop=(kc2 == 1 and k == KO - 1))
                        if half == 0:
                            # acc = psum + residual (attn_out)
                            nc.vector.tensor_add(acc[:, jc, :], ps[:],
                                                 attnout_sb[:, jc, bsl])
                        else:
                            t = pool.tile([128, 512], F32, tag="misc", bufs=3,
                                          name=f"ft_{b}_{jc}")
                            nc.vector.tensor_add(t[:], ps[:], acc[:, jc, :])
                            ot = pool.tile([128, 512], F32, tag="outs", bufs=3,
                                           name=f"ot_{b}_{jc}")
                            nc.scalar.activation(ot[:], t[:], AF.Identity,
                                                 bias=bdn_sb[:, jc:jc + 1])
                            nc.sync.dma_start(outT[jc, :, bsl], ot[:])

    nc.compile()
    return nc


_NC = None


def _get_nc():
    global _NC
    if _NC is None:
        _NC = _build()
    return _NC


def _prepare_in_maps(inputs):
    f32 = lambda a: np.ascontiguousarray(np.asarray(a, dtype=np.float32))

    x = f32(inputs["hidden_states"])            # [B,S,H]
    shared = {
        "wq": f32(inputs["Wq"]).reshape(KO, 128, H),
        "wk": f32(inputs["Wk"]).reshape(KO, 128, H),
        "wv": f32(inputs["Wv"]).reshape(KO, 128, H),
        "wo": f32(inputs["Wo"]).reshape(KO, 128, H),
        "bq": f32(inputs["bq"]).reshape(KO, 128),
        "bk": f32(inputs["bk"]).reshape(KO, 128),
        "bvr": f32(inputs["bv"]).reshape(1, H),
        "bo": f32(inputs["bo"]).reshape(KO, 128),
        "g1": f32(inputs["ln1_g"]).reshape(KO, 128),
        "be1": f32(inputs["ln1_b"]).reshape(KO, 128),
        "g2": f32(inputs["ln2_g"]).reshape(KO, 128),
        "be2": f32(inputs["ln2_b"]).reshape(KO, 128),
        "gate": f32(inputs["gate_W"]).reshape(KO, 128, E),
        "wup": f32(inputs["W_up"]).reshape(E, KO, 128, I),
        "wdn": f32(inputs["W_down"]).reshape(E, IO, 128, H),
        "bup": f32(inputs["b_up"]).reshape(E, IO, 128),
        "bdn": f32(inputs["b_down"]).reshape(E, KO, 128),
    }
    cstv = np.zeros((128, 160), dtype=np.float32)
    cstv[:, 0:8] = 1.0
    cstv[:, 8:16] = 1.0 / H
    cstv[:, 24:152] = 1.0
    shared["cst"] = cstv
    shared["rev"] = np.arange(E, 0, -1, dtype=np.float32).reshape(1, E)

    in_maps = []
    for c in range(N_CORES):
        xc = x[c * BPC:(c + 1) * BPC]           # [2,S,H]
        xT_c = np.ascontiguousarray(
            xc.transpose(2, 0, 1).reshape(KO, 128, BPC * S))
        m = dict(shared)
        m["xT"] = xT_c
        in_maps.append(m)
    return in_maps


def _postprocess(results):
    out = np.empty((B, S, H), dtype=np.float32)
    logits = np.empty((B, E), dtype=np.float32)
    for c in range(N_CORES):
        oT = results[c]["outT"]                 # [KO,128,BPC*S]
        out[c * BPC:(c + 1) * BPC] = (
            oT.reshape(KO * 128, BPC, S).transpose(1, 2, 0))
        logits[c * BPC:(c + 1) * BPC] = results[c]["lg_out"]
    return out, logits


def kernel(**inputs):
    nc = _get_nc()
    in_maps = _prepare_in_maps(inputs)
    res = run_bass_kernel_spmd(nc, in_maps, list(range(N_CORES)))
    return _postprocess(res.results)
